# revision 8
# baseline (speedup 1.0000x reference)
"""Adaptive-softmax loss (nn_AdaptiveLoss) on 8 trn2 NeuronCores.

Strategy: tensor-parallel over the vocab dimension, 8-way. Each core owns
1/8 of the shortlist head columns and 1/8 of each tail cluster's output
rows. Per core:

  - computes cluster hidden states h_g = x @ proj_g.T (replicated, small)
    with fp8 DoubleRow matmuls; h0/h1 requantized to fp8, h2/h3 to bf16,
  - computes its slice of every group's logits: fp8 DoubleRow for the
    K>=256 groups (head/c0/c1), bf16 for the K<=128 clusters (c2/c3);
    weights are prescaled so every group's PSUM logit carries the same
    x64 factor, folded back out via the exp activation scale,
  - exp()s the logits on ACT in 7 PSUM pieces per 128-row tile, with the
    ACT accumulator giving per-piece sums; per-group softmax denominators
    are reconstructed from piece sums plus four narrow boundary sums on
    DVE,
  - gathers exp(logit) at this core's share of the targets straight out
    of SBUF (gpsimd indirect_copy, kept awake by dummy gathers before the
    final row tile) and takes ln in-loop,
  - per-row weight sums (den, W_g) are input-only quantities computed on
    the host and shipped as small tensors - they never ride a collective,
  - the per-row partial stats (5 softmax Zs + weighted-logit numerator,
    [128, 48] fp32) are exchanged with the 7 peers by direct remote SBUF
    DMA writes (XOR-slot all-gather, descriptors generated early and
    triggered at the end), then every core sums the 8 slots and finishes
    the cheap log/normalize arithmetic identically.

The full [B, VOCAB] log-prob matrix is never materialized anywhere, and
no ncfw collective is on the critical path (only the kernel-entry
barrier AllGather, which completes long before the tail needs it).
"""

import sys

sys.path.insert(0, "/opt/trn_rl_repo")

from contextlib import ExitStack

import ml_dtypes
import numpy as np

import concourse.bass as bass  # noqa: F401  (engine types via nc.*)
import concourse.mybir as mybir
import concourse.tile as tile
from concourse import bacc
from concourse.bass_utils import run_bass_kernel_spmd

BF16 = ml_dtypes.bfloat16
F8 = ml_dtypes.float8_e4m3
F32 = mybir.dt.float32
BF16_DT = mybir.dt.bfloat16
F8_DT = mybir.dt.float8e4
U16 = mybir.dt.uint16

NCORES = 8
B, T, D = 1024, 128, 1024
VOCAB, SHORT = 100000, 10000
CL_SIZES = [10000, 20000, 40000, 20000]
CL_D = [512, 256, 128, 64]
SH_SHARD = SHORT // NCORES                      # 1250
CL_SHARD = [s // NCORES for s in CL_SIZES]      # 1250 2500 5000 2500
GRP_BOUNDS = [0, 10000, 20000, 40000, 80000, 100000]
GRP_SHARD = [SH_SHARD] + CL_SHARD

# per-core concatenated logits layout: [head | links(4) | c0 | c1 | c2 | c3]
OFF_HEAD = 0
OFF_LINK = SH_SHARD                              # 1250
OFF_CL = [1254, 2504, 5004, 10004]
GRP_OFF = [OFF_HEAD] + OFF_CL                    # per-group concat offset
CONCAT = OFF_CL[-1] + CL_SHARD[-1]               # 12504
CONCAT_PAD = 12544
# pad slots gather column 0 (always computed, finite); their wm==0 makes
# the contribution vanish.
PADIDX = 0
RT = 8                                           # row tiles of 128

# fp8 scale factors (folded back out via the exp activation scale)
S_WHEAD = 64.0                                   # head weight prescale
S_PROJ = 32.0                                    # proj prescale -> h scale
S_WOUT = 2.0                                     # c0/c1 out-proj prescale
S_W23 = 2.0                                      # c2/c3 out-proj prescale
EXP_TABLE_ID = 6                                 # natural_log_exp_and_others
import os as _os
FP8_ON = not _os.environ.get("ADAK_BF16")

# PSUM piece bounds (7 pieces, 2048-wide except the tail)
PB = [0, 2048, 4096, 6144, 8192, 10240, 12288, CONCAT]
NPIECE = 7
# (lo, hi) of the boundary small-side sums, their sv slot = index
SVS = [(1254, 2048), (2048, 2504), (4096, 5004), (10004, 10240)]
# payload stats per row: q = 0 Zh, 1..4 Zc_g, 5 numraw
NSTAT = 6
PAYW = NSTAT * RT                                # 48


# ----------------------------------------------------------------------------
# device kernel builder
# ----------------------------------------------------------------------------

_CACHE: dict[int, object] = {}


def _build(S: int):
    """Build + compile the SPMD kernel for padded slot count S (multiple of 16)."""
    if S in _CACHE:
        return _CACHE[S]
    SW = S // 16

    nc = bacc.Bacc("TRN2", target_bir_lowering=False, debug=False,
                   num_devices=NCORES)

    MMDT = F8_DT if FP8_ON else BF16_DT
    xt_d = nc.dram_tensor("xt", [D, B], MMDT, kind="ExternalInput")
    projt_d = nc.dram_tensor("projt", [D, sum(CL_D)], MMDT, kind="ExternalInput")
    whead_d = nc.dram_tensor("wheadt", [D, 1254], MMDT, kind="ExternalInput")
    wout0_d = nc.dram_tensor("wout0t", [CL_D[0], CL_SHARD[0]], MMDT,
                             kind="ExternalInput")
    wout1_d = nc.dram_tensor("wout1t", [CL_D[1], CL_SHARD[1]], MMDT,
                             kind="ExternalInput")
    wout2_d = nc.dram_tensor("wout2t", [CL_D[2], CL_SHARD[2]], BF16_DT,
                             kind="ExternalInput")
    wout3_d = nc.dram_tensor("wout3t", [CL_D[3], CL_SHARD[3]], BF16_DT,
                             kind="ExternalInput")
    tix_d = nc.dram_tensor("tgtidx", [128, RT * SW], U16, kind="ExternalInput")
    wm_d = nc.dram_tensor("wm", [128, RT, S], BF16_DT, kind="ExternalInput")
    den_d = nc.dram_tensor("den", [128, RT], F32, kind="ExternalInput")
    rden_d = nc.dram_tensor("rden", [128, RT], F32, kind="ExternalInput")
    wgq_d = nc.dram_tensor("wgq", [128, 4, RT], F32, kind="ExternalInput")
    out_d = nc.dram_tensor("out", [1, 1], F32, kind="ExternalOutput")
    DBG = bool(_os.environ.get("ADAK_DBG"))
    if DBG:
        pay_d = nc.dram_tensor("pay_dump", [128, PAYW], F32,
                               kind="ExternalOutput")
        rsum_d = nc.dram_tensor("rsum_dump", [128, PAYW], F32,
                                kind="ExternalOutput")
        zcomb_d = nc.dram_tensor("zcomb_dump", [128, 40], F32,
                                 kind="ExternalOutput")

    EXP = mybir.ActivationFunctionType.Exp
    LN = mybir.ActivationFunctionType.Ln
    ADD = mybir.AluOpType.add
    SUB = mybir.AluOpType.subtract
    MULT = mybir.AluOpType.mult
    AXX = mybir.AxisListType.X
    DR = mybir.MatmulPerfMode.DoubleRow

    with tile.TileContext(nc) as tc, ExitStack() as ctx:
        sb = ctx.enter_context(tc.tile_pool(name="sb", bufs=1))
        big = ctx.enter_context(tc.tile_pool(name="big", bufs=3))
        ps = ctx.enter_context(tc.tile_pool(name="ps", bufs=2, space="PSUM"))

        # combined exp+ln activation table so EXP and LN interleave with a
        # single table load for the whole kernel
        import os
        if not os.environ.get("ADAK_NO_TABLE_PRELOAD"):
            nc.scalar.add_instruction(mybir.InstLoadActFuncSet(
                name=nc.get_next_instruction_name(),
                act_func_set_id=EXP_TABLE_ID, ins=[], outs=[]))

        # ---- persistent SBUF tensors ----
        xt_sb = sb.tile([128, 8, B], MMDT)             # x.T  [d, b] k-tiled
        whead_sb = sb.tile([128, 8, 1254], MMDT)
        wout0_sb = sb.tile([128, 4, CL_SHARD[0]], MMDT)
        wout1_sb = sb.tile([128, 2, CL_SHARD[1]], MMDT)
        wout2_sb = sb.tile([128, CL_SHARD[2]], BF16_DT)
        wout3_sb = sb.tile([64, CL_SHARD[3]], BF16_DT)
        h0_sb = sb.tile([128, 4, B], MMDT)             # h.T (x S_PROJ)
        h1_sb = sb.tile([128, 2, B], MMDT)
        h2_sb = sb.tile([128, B], BF16_DT)
        h3_sb = sb.tile([64, B], BF16_DT)
        tix_sb = sb.tile([128, RT * SW], U16)
        vg3 = sb.tile([128, RT, S], BF16_DT)           # gathered exp(logit)
        wm_sb = sb.tile([128, RT, S], BF16_DT)         # (1-dp)*ownership
        logv3 = sb.tile([128, RT, S], BF16_DT)
        llinkraw = sb.tile([128, RT, 4], F32)          # raw link logits
        zscr = sb.tile([128, 2048], BF16_DT)
        zs = sb.tile([128, RT, NPIECE], F32)  # per-piece exp-sum accumulators
        sv = sb.tile([128, RT, 4], F32)       # boundary small-side sums
        pay = sb.tile([128, PAYW], F32)       # per-core stats payload
        rgath = sb.tile([128, 8, PAYW], F32)  # peer payload gather slots
        rsum = sb.tile([128, PAYW], F32)
        den_sb = sb.tile([128, RT], F32)
        rden_sb = sb.tile([128, RT], F32)
        wgq_sb = sb.tile([128, 4, RT], F32)
        ones_sb = sb.tile([128, 1], F32)
        warmg = sb.tile([128, 16], BF16_DT)   # gpsimd keep-awake gather dst
        zix = sb.tile([128, 1], U16)
        out_sb = sb.tile([1, 1], F32)

        pview = pay[:, :].rearrange("p (q r) -> p q r", q=NSTAT)
        rsq = rsum[:, :].rearrange("p (q r) -> p q r", q=NSTAT)

        # ---- remote all-gather plumbing: clear the handshake sems before
        # any peer can possibly send (their sends sit behind the kernel
        # entry barrier + ~160us of compute) ----
        rsem = nc.alloc_semaphore("adak_rsem")
        lsem = nc.alloc_semaphore("adak_lsem")
        psem = nc.alloc_semaphore("adak_psem")
        with tc.tile_critical():
            nc.gpsimd.sem_clear(rsem)
            nc.gpsimd.sem_clear(lsem)
            nc.gpsimd.sem_clear(psem)

        # ---- input DMAs (order matters: compute-critical tensors first;
        # xt/projt interleaved per k-tile so the h matmuls start early) ----
        pj = sb.tile([128, 8, sum(CL_D)], MMDT)
        xt_r = xt_d.ap().rearrange("(k p) b -> p k b", p=128)
        pj_r = projt_d.ap().rearrange("(k p) c -> p k c", p=128)
        wh_r = whead_d.ap().rearrange("(k p) c -> p k c", p=128)
        for k in range(8):
            nc.sync.dma_start(out=xt_sb[:, k, :], in_=xt_r[:, k, :])
            nc.sync.dma_start(out=whead_sb[:, k, :], in_=wh_r[:, k, :])
        for k in range(8):
            nc.sync.dma_start(out=pj[:, k, :], in_=pj_r[:, k, :])
        nc.sync.dma_start(out=wout0_sb,
                          in_=wout0_d.ap().rearrange("(k p) c -> p k c", p=128))
        nc.sync.dma_start(out=wout1_sb,
                          in_=wout1_d.ap().rearrange("(k p) c -> p k c", p=128))
        nc.sync.dma_start(out=wout2_sb, in_=wout2_d[:])
        nc.sync.dma_start(out=wout3_sb, in_=wout3_d[:])
        nc.sync.dma_start(out=tix_sb, in_=tix_d[:])
        nc.sync.dma_start(out=wm_sb, in_=wm_d[:])
        nc.sync.dma_start(out=den_sb, in_=den_d[:])
        nc.sync.dma_start(out=rden_sb, in_=rden_d[:])
        nc.sync.dma_start(out=wgq_sb, in_=wgq_d[:])

        nc.vector.memset(ones_sb[:, :], 1.0)
        nc.vector.memset(zix[:, :], 0)

        # ---- cluster hidden states h.T (all batch rows, computed locally) --
        HT_OFF = [0, 128, 256, 384, 512, 640, 768, 896]
        HT_M = [128, 128, 128, 128, 128, 128, 128, 64]

        def emit_h():
          for bc in range(2):
              for htile in range(2):
                  pst = ps.tile([128, 2048], F32, tag="ps", name=f"hps_{bc}_{htile}")
                  for hl in range(4):
                      ht = htile * 4 + hl
                      M = HT_M[ht]
                      if FP8_ON:
                          for kp in range(4):
                              nc.tensor.matmul(
                                  pst[0:M, hl * 512:(hl + 1) * 512],
                                  pj[:, 2 * kp:2 * kp + 2, HT_OFF[ht]:HT_OFF[ht] + M],
                                  xt_sb[:, 2 * kp:2 * kp + 2, bc * 512:(bc + 1) * 512],
                                  start=(kp == 0), stop=(kp == 3), perf_mode=DR)
                      else:
                          for k in range(8):
                              nc.tensor.matmul(
                                  pst[0:M, hl * 512:(hl + 1) * 512],
                                  pj[:, k, HT_OFF[ht]:HT_OFF[ht] + M],
                                  xt_sb[:, k, bc * 512:(bc + 1) * 512],
                                  start=(k == 0), stop=(k == 7))
                  for hl in range(4):
                      ht = htile * 4 + hl
                      src = pst[0:HT_M[ht], hl * 512:(hl + 1) * 512]
                      bsl = slice(bc * 512, (bc + 1) * 512)
                      if ht < 4:
                          nc.scalar.copy(h0_sb[:, ht, bsl], src)
                      elif ht < 6:
                          nc.vector.tensor_copy(h1_sb[:, ht - 4, bsl], src)
                      elif ht == 6:
                          nc.vector.tensor_copy(h2_sb[:, bsl], src)
                      else:
                          nc.vector.tensor_copy(h3_sb[0:64, bsl], src)

        # ---- main loop: logits -> exp (+Z accumulate) -> gather/ln --------
        KW = 2 if FP8_ON else 1

        def lh_head(kp, rt):
            return xt_sb[:, KW * kp:KW * kp + KW, rt * 128:(rt + 1) * 128]

        def lh_c0(kp, rt):
            return h0_sb[:, KW * kp:KW * kp + KW, rt * 128:(rt + 1) * 128]

        def lh_c1(kp, rt):
            return h1_sb[:, KW * kp:KW * kp + KW, rt * 128:(rt + 1) * 128]

        def lh_c2(kp, rt):
            return h2_sb[:, rt * 128:(rt + 1) * 128]

        def lh_c3(kp, rt):
            return h3_sb[0:64, rt * 128:(rt + 1) * 128]

        def rh_head(kp, a, w):
            return whead_sb[:, KW * kp:KW * kp + KW, a:a + w]

        def rh_w0(kp, a, w):
            return wout0_sb[:, KW * kp:KW * kp + KW, a:a + w]

        def rh_w1(kp, a, w):
            return wout1_sb[:, KW * kp:KW * kp + KW, a:a + w]

        def rh_w2(kp, a, w):
            return wout2_sb[:, a:a + w]

        def rh_w3(kp, a, w):
            return wout3_sb[0:64, a:a + w]

        ESC = 1.0 / S_WHEAD                # uniform: all logits land x64
        if FP8_ON:
            GROUPS = [
                (0, 1254, 4, True, lh_head, rh_head),
                (OFF_CL[0], 1250, 2, True, lh_c0, rh_w0),
                (OFF_CL[1], 2500, 1, True, lh_c1, rh_w1),
                (OFF_CL[2], 5000, 1, False, lh_c2, rh_w2),
                (OFF_CL[3], 2500, 1, False, lh_c3, rh_w3),
            ]
        else:
            GROUPS = [
                (0, 1254, 8, False, lh_head, rh_head),
                (OFF_CL[0], 1250, 4, False, lh_c0, rh_w0),
                (OFF_CL[1], 2500, 2, False, lh_c1, rh_w1),
                (OFF_CL[2], 5000, 1, False, lh_c2, rh_w2),
                (OFF_CL[3], 2500, 1, False, lh_c3, rh_w3),
            ]

        # piece emission order for non-hoisted row tiles: tiny/cluster pieces
        # first so ACT has food while the PE-heavy head piece streams
        ORDER = [6, 3, 0, 2, 1, 4, 5]

        t8z = sb.tile([128, 8], F32)
        linkexp = sb.tile([128, 32], F32)
        lsum = sb.tile([128, 8], F32)

        # piece 0 now spans c0 columns (which need h0), so no piece can be
        # hoisted ahead of emit_h; ACT stays busy there with the h0 copies.
        HOIST = 0

        def emit_piece(rt, pi, expb):
            lo, hi = PB[pi], PB[pi + 1]
            pst = ps.tile([128, hi - lo], F32, tag="ps",
                          name=f"ps_{rt}_{pi}")
            for goff, width, kt, fp8, lh, rh in GROUPS:
                slo, shi = max(goff, lo), min(goff + width, hi)
                if slo >= shi:
                    continue
                subs = []
                a = slo
                while a < shi:
                    w = min(shi - a, 512 - ((a - lo) % 512))
                    subs.append((a, w))
                    a += w
                for kp in range(kt):
                    for a, w in subs:
                        nc.tensor.matmul(
                            pst[:, a - lo:a - lo + w],
                            lh(kp, rt), rh(kp, a - goff, w),
                            start=(kp == 0), stop=(kp == kt - 1),
                            perf_mode=DR if fp8 else None)
            nc.scalar.activation(
                expb[:, lo:hi], pst[:, 0:hi - lo], EXP,
                scale=ESC, accum_out=zs[:, rt, pi:pi + 1])
            if pi == 0:
                # raw link logits out of PSUM (their ln IS the logit)
                nc.vector.tensor_scalar(
                    llinkraw[:, rt, :], pst[:, 1250:1254],
                    ESC, None, op0=MULT)

        def emit_zfix(r0, r1):
            # reconstruct per-group Z from piece accumulators + boundary
            # sums for row tiles [r0, r1); Zh still needs the link exp sum
            # subtracted once at the end.
            sl = slice(r0, r1)
            # Zh(partial) = A0 - sv0
            nc.vector.tensor_tensor(pview[:, 0, sl], zs[:, sl, 0],
                                    sv[:, sl, 0], SUB)
            # Zc0 = sv0 + sv1
            nc.vector.tensor_tensor(pview[:, 1, sl], sv[:, sl, 0],
                                    sv[:, sl, 1], ADD)
            # Zc1 = A1 - sv1 + sv2
            nc.vector.tensor_tensor(t8z[:, sl], zs[:, sl, 1], sv[:, sl, 1], SUB)
            nc.vector.tensor_tensor(pview[:, 2, sl], t8z[:, sl],
                                    sv[:, sl, 2], ADD)
            # Zc2 = A2 - sv2 + A3 + A4 - sv3
            nc.vector.tensor_tensor(t8z[:, sl], zs[:, sl, 2], sv[:, sl, 2], SUB)
            nc.vector.tensor_tensor(t8z[:, sl], t8z[:, sl], zs[:, sl, 3], ADD)
            nc.vector.tensor_tensor(t8z[:, sl], t8z[:, sl], zs[:, sl, 4], ADD)
            nc.vector.tensor_tensor(pview[:, 3, sl], t8z[:, sl],
                                    sv[:, sl, 3], SUB)
            # Zc3 = sv3 + A5 + A6
            nc.vector.tensor_tensor(t8z[:, sl], sv[:, sl, 3], zs[:, sl, 5], ADD)
            nc.vector.tensor_tensor(pview[:, 4, sl], t8z[:, sl],
                                    zs[:, sl, 6], ADD)

        tmp2S = sb.tile([128, 2, S], BF16_DT)

        def emit_numer(r0, r1):
            # ln of gathered exp values + weighted-sum numerator for row
            # tiles [r0, r1) (tensor_tensor_reduce faults on hw — avoid)
            n = r1 - r0
            nc.scalar.activation(
                logv3[:, r0:r1, :].rearrange("p a b -> p (a b)"),
                vg3[:, r0:r1, :].rearrange("p a b -> p (a b)"), LN)
            nc.vector.tensor_tensor(
                tmp2S[:, 0:n, :], logv3[:, r0:r1, :], wm_sb[:, r0:r1, :], MULT)
            nc.vector.tensor_reduce(
                pview[:, 5, r0:r1], tmp2S[:, 0:n, :], AXX, ADD)

        expbs = {}
        for rt in range(HOIST):
            expbs[rt] = big.tile([128, CONCAT_PAD], BF16_DT, tag="big",
                                 name=f"expb_{rt}")
            emit_piece(rt, 0, expbs[rt])

        emit_h()

        for rt in range(RT):
            expb = expbs.get(rt)
            if expb is None:
                expb = big.tile([128, CONCAT_PAD], BF16_DT, tag="big",
                                name=f"expb_{rt}")
            for oi, pi in enumerate(range(1, NPIECE) if rt < HOIST else ORDER):
                emit_piece(rt, pi, expb)
                if rt == RT - 1 and pi in (2, 4, 5):
                    # keep the gpsimd Q7 awake so the final gather doesn't
                    # pay its ~8us wake latency
                    nc.gpsimd.indirect_copy(
                        warmg[:, 0:16], expb[:, PB[pi]:PB[pi] + 16],
                        zix[:, 0:1], True)
                if oi == 4 and rt >= 2 and rt % 2 == 0:
                    emit_numer(rt - 2, rt)
                if oi == 4 and rt == RT - 1:
                    emit_numer(RT - 2, RT - 1)
            # boundary small-side sums on DVE (link cols excluded from sv0)
            for q, (za, zb) in enumerate(SVS):
                nc.vector.tensor_scalar(
                    zscr[:, 0:zb - za],
                    expb[:, za:zb], 1.0, 0.0, op0=MULT, op1=ADD,
                    accum_out=sv[:, rt, q:q + 1])
            if rt == 3 or rt == RT - 1:
                emit_zfix(0 if rt == 3 else 4, rt + 1)
            # gather exp(logit) at this core's targets
            nc.gpsimd.indirect_copy(
                vg3[:, rt, :], expb[:, 0:CONCAT],
                tix_sb[:, rt * SW:(rt + 1) * SW], True)
        emit_numer(RT - 1, RT)

        # link exp sums: computed once, subtracted from the Zh partials
        # (links are replicated on every core; added back exactly once
        # after the cross-core sum)
        nc.scalar.activation(
            linkexp[:, :],
            llinkraw[:, :, :].rearrange("p a b -> p (a b)"), EXP)
        lx3 = linkexp[:, :].rearrange("p (r g) -> p r g", g=4)
        nc.vector.tensor_reduce(lsum[:, :], lx3, AXX, ADD)
        nc.vector.tensor_tensor(pview[:, 0, :], pview[:, 0, :], lsum[:, :], SUB)

        # self slot of the all-gather
        nc.vector.tensor_copy(rgath[:, 0, :], pay[:, :])

        # ---- fire the peer writes, wait for all 7 peers, sum the slots ----
        with tc.tile_critical():
            for k in range(1, 8):
                rdests = [(0, k) if j == k else None for j in range(8)]
                nc.gpsimd.remote_dma_broadcast(
                    rgath[:, k, :], pay[:, :], rsem, lsem,
                    rdests=rdests).then_inc(psem, 1)
            nc.gpsimd.wait_ge(psem, 7)
            nc.gpsimd.bir_kernel_barrier_wait([list(range(NCORES))])
            nc.gpsimd.trigger_dma(count=7)
            nc.vector.wait_ge(rsem, 14)
            for k in range(1, 8):
                nc.vector.tensor_tensor(rsum[:, :],
                                        rgath[:, 0, :] if k == 1 else rsum[:, :],
                                        rgath[:, k, :], ADD)

        # ---- final combine (identical on every core) ----
        zcomb = sb.tile([128, 40], F32)
        lnz = sb.tile([128, 40], F32)
        s8 = sb.tile([128, 8], F32)
        tA = sb.tile([128, 8], F32)
        num8 = sb.tile([128, 8], F32)
        pcol = sb.tile([128, 1], F32)
        llview = llinkraw[:, :, :]

        nc.vector.tensor_tensor(zcomb[:, 0:8], rsq[:, 0, :], lsum[:, :], ADD)
        nc.vector.tensor_copy(zcomb[:, 8:40], rsum[:, 8:40])
        nc.scalar.activation(lnz[:, :], zcomb[:, :], LN)
        llink3 = llview.rearrange("p r g -> p g r")
        for g in range(4):
            nc.vector.tensor_tensor(
                tA[:, :], llink3[:, g, :], lnz[:, 8 + 8 * g:16 + 8 * g], SUB)
            if g == 0:
                nc.vector.tensor_tensor(s8[:, :], tA[:, :], wgq_sb[:, g, :], MULT)
            else:
                nc.vector.tensor_tensor(tA[:, :], tA[:, :], wgq_sb[:, g, :], MULT)
                nc.vector.tensor_tensor(s8[:, :], s8[:, :], tA[:, :], ADD)
        # num = numraw + s8 - den * logZh, scaled by 1/den
        nc.vector.tensor_tensor(tA[:, :], den_sb[:, :], lnz[:, 0:8], MULT)
        nc.vector.tensor_tensor(num8[:, :], rsq[:, 5, :], tA[:, :], SUB)
        nc.vector.tensor_tensor(num8[:, :], num8[:, :], s8[:, :], ADD)
        nc.vector.tensor_tensor(num8[:, :], num8[:, :], rden_sb[:, :], MULT)
        nc.vector.tensor_reduce(pcol[:, :], num8[:, :], AXX, ADD)
        psq = ps.tile([1, 1], F32, tag="ps")
        nc.tensor.matmul(psq[0:1, 0:1], pcol[:, 0:1], ones_sb[:, 0:1],
                         start=True, stop=True)
        nc.scalar.mul(out_sb[:, :], psq[0:1, 0:1], -1.0 / (B + 1e-5))
        nc.sync.dma_start(out=out_d[:], in_=out_sb)
        if DBG:
            nc.sync.dma_start(out=pay_d[:], in_=pay[:, :])
            nc.sync.dma_start(out=rsum_d[:], in_=rsum[:, :])
            nc.sync.dma_start(out=zcomb_d[:], in_=zcomb[:, :])

    nc.compile()
    _CACHE[S] = nc
    return nc


# ----------------------------------------------------------------------------
# host-side sharding / index routing
# ----------------------------------------------------------------------------


def _f8(a, scale):
    return np.clip(np.asarray(a, np.float32) * scale, -239.0, 239.0).astype(F8)


def _shard_inputs(features, head_weight, projs, outs, discard_probs,
                  targets, target_mask):
    """Build the 8 per-core input maps. Returns (in_maps, S)."""
    if FP8_ON:
        xt = _f8(np.ascontiguousarray(features.T), 1.0)
        projt = _f8(np.concatenate([p.T for p in projs], axis=1), S_PROJ)
    else:
        xt = np.ascontiguousarray(features.T).astype(BF16)
        projt = (np.concatenate([p.T for p in projs], axis=1)
                 * S_PROJ).astype(BF16)

    tgt = np.asarray(targets).astype(np.int64).reshape(-1)
    msk = np.asarray(target_mask).astype(bool).reshape(-1)
    bb = np.repeat(np.arange(B, dtype=np.int64), T)

    grp = np.digitize(tgt, GRP_BOUNDS[1:-1])          # 0..4 (0 = shortlist)
    u = tgt - np.asarray(GRP_BOUNDS)[grp]
    shard = np.asarray(GRP_SHARD)[grp]
    core = u // shard
    jcat = u % shard + np.asarray(GRP_OFF)[grp]
    wval = (1.0 - discard_probs[tgt]).astype(np.float32)

    rt = bb >> 7
    gc = (bb >> 4) & 7

    # per-row weight sums: input-only, computed here instead of on-device
    wv = wval * msk
    den_row = np.bincount(bb, weights=wv, minlength=B).astype(np.float32)
    wg_row = np.zeros((B, 4), np.float32)
    for g in range(1, 5):
        selg = grp == g
        wg_row[:, g - 1] = np.bincount(bb[selg], weights=wv[selg],
                                       minlength=B)
    den_in = den_row.reshape(RT, 128).T.copy()            # [p, rt]
    rden_in = (1.0 / np.maximum(den_row, 1e-20)).reshape(RT, 128).T.copy()
    wgq_in = np.ascontiguousarray(
        wg_row.reshape(RT, 128, 4).transpose(1, 2, 0))    # [p, g, rt]

    # padded slots per (core, rt, gc)
    key_all = ((core * RT + rt) * 8 + gc).astype(np.int64)
    valid = msk
    counts = np.bincount(key_all[valid], minlength=NCORES * RT * 8)
    # multiple of 32 so each row-tile's wrapped idx slice stays 4B-aligned
    S = int(counts.max())
    S = ((S + 31) // 32) * 32

    in_maps = []
    for c in range(NCORES):
        sel = valid & (core == c)
        jj = jcat[sel]
        bsel = bb[sel]
        rts = rt[sel]
        gcs = gc[sel]
        ww = wval[sel]
        po = bsel & 15
        key = rts * 8 + gcs
        order = np.argsort(key, kind="stable")
        jj, bsel, rts, gcs, po, ww = (a[order] for a in
                                      (jj, bsel, rts, gcs, po, ww))
        key = key[order]
        # slot within each (rt, gc) bucket
        start_of = np.r_[0, np.flatnonzero(np.diff(key)) + 1]
        bucket_len = np.diff(np.r_[start_of, len(key)])
        slot = np.arange(len(key)) - np.repeat(start_of, bucket_len)

        tix = np.full((128, RT * (S // 16)), PADIDX, np.uint16)
        tix[16 * gcs + slot % 16, rts * (S // 16) + slot // 16] = jj.astype(np.uint16)
        wm = np.zeros((128, RT, S), np.float32)
        wm[16 * gcs + po, rts, slot] = ww
        wm = wm.astype(BF16)

        # head shard + link columns, transposed
        hslice = head_weight[c * SH_SHARD:(c + 1) * SH_SHARD]
        wh_cat = np.concatenate(
            [hslice.T, head_weight[SHORT:SHORT + 4].T], axis=1)
        wheadt = (_f8(wh_cat, S_WHEAD) if FP8_ON
                  else (wh_cat * S_WHEAD).astype(BF16))
        in_maps.append({
            "xt": xt,
            "projt": projt,
            "wheadt": wheadt,
            "wout0t": (_f8(outs[0][c * CL_SHARD[0]:(c + 1) * CL_SHARD[0]].T,
                           S_WOUT) if FP8_ON else
                       (outs[0][c * CL_SHARD[0]:(c + 1) * CL_SHARD[0]].T
                        * S_WOUT).astype(BF16)),
            "wout1t": (_f8(outs[1][c * CL_SHARD[1]:(c + 1) * CL_SHARD[1]].T,
                           S_WOUT) if FP8_ON else
                       (outs[1][c * CL_SHARD[1]:(c + 1) * CL_SHARD[1]].T
                        * S_WOUT).astype(BF16)),
            "wout2t": np.ascontiguousarray(
                outs[2][c * CL_SHARD[2]:(c + 1) * CL_SHARD[2]].T
                * S_W23).astype(BF16),
            "wout3t": np.ascontiguousarray(
                outs[3][c * CL_SHARD[3]:(c + 1) * CL_SHARD[3]].T
                * S_W23).astype(BF16),
            "tgtidx": tix,
            "wm": wm,
            "den": den_in,
            "rden": rden_in,
            "wgq": wgq_in,
        })
    return in_maps, S


def _run(features, head_weight, proj0, out0, proj1, out1, proj2, out2,
         proj3, out3, discard_probs, targets, target_mask,
         trace=False, tmpdir=None):
    features = np.asarray(features, np.float32)
    head_weight = np.asarray(head_weight, np.float32)
    projs = [np.asarray(p, np.float32) for p in (proj0, proj1, proj2, proj3)]
    outs = [np.asarray(o, np.float32) for o in (out0, out1, out2, out3)]
    discard_probs = np.asarray(discard_probs, np.float32)

    in_maps, S = _shard_inputs(features, head_weight, projs, outs,
                               discard_probs, targets, target_mask)
    nc = _build(S)
    res = run_bass_kernel_spmd(nc, in_maps, list(range(NCORES)),
                               trace=trace, tmpdir=tmpdir)
    val = np.asarray(res.results[0]["out"], np.float32).reshape(())
    return val, res


def kernel(**inputs) -> np.ndarray:
    val, _ = _run(**inputs)
    return val
